# revision 32
# baseline (speedup 1.0000x reference)
"""Trainium2 Bass kernel for the CaputoEncoder model.

Model (see reference): feats = concat([caputo(x, 0.5), caputo(x, 1.0)], -1)
-> 2-layer LSTM(512) -> last timestep -> relu(linear).

Key simplifications:
  * caputo(x, 1.0) has coefficient 1/gamma(0) == 0 -> contributes zeros;
    only the alpha=0.5 branch matters, so only Wih0[:, :250] is ever used.
  * caputo(x, .5) = d*x - Wc@x (over time) == G @ x_b with G = diag(d) - Wc,
    host-precomputed; becomes a single matmul per batch.

Sharding: pure data parallelism over batch (64 -> 8 per core, 8 cores).
All weights replicated; scatter/gather on host.

Layer pipelining: the two LSTM scans run interleaved step-by-step, layer 1
one window (32 steps) behind layer 0, in a 2x-unrolled hardware loop.  Per
step the PE runs 1 identity-matmul (injects xw_t into PSUM) + 64 Whh
matmuls per layer; emission order is gates(L0), gates(L1), cell(L0),
cell(L1) so each layer's cell chain hides under the other layer's matmul
burst.  xw1 = A1 @ h0-window stays SBUF-resident (stgA/stgB ping-pong, no
DRAM round-trip); xw0 windows are pair-loaded per loop body (a single
dynamic-offset DMA group per body — two break AP lowering).

Cell chain (per layer, 3+1 ACT + 4 DVE ops): with tanh(x) = 2*sigmoid(2x)-1
and h~ := h/2 stored everywhere (consumer weights pre-doubled host-side,
g-gate rows pre-scaled 2x):
    g/i,f/o = sigmoid(psum cols)          # ACT, split so g starts ~25% into
                                          # the burst, i/f at ~75% (partial-
                                          # psum region deps)
    g~   = sig_g - 0.5                    # DVE tensor_scalar
    prods= [i|f] * [g~|c_prev]            # DVE, one 64-col multiply
    c    = 2*prods_ig + prods_fc          # DVE scalar_tensor_tensor
    tc   = sigmoid(2*c)                   # ACT
    h~   = (tc - 0.5) * o                 # DVE scalar_tensor_tensor
Ordering-only dep edges (tc before next sigmoid on ACT, h~ before next
tensor_scalar on DVE) pin the engine FIFO order the Tile scheduler would
otherwise break.  Full-bank psum tiles prevent start=True has_written
clears from serializing against the other pool's readers.

On-core layout (hidden-major):
  h~, c   : (128 part = hidden%128, cols = kchunk*8 + b)   [4*8=32 cols]
  gatesT  : (128 part = gate%128,  cols = gchunk*8 + b)    [16*8=128 cols]
  gate chunks host-permuted to [g, i, f, o].
"""

import math
from contextlib import ExitStack

import numpy as np
import ml_dtypes

import concourse.bass as bass
import concourse.tile as tile
from concourse import mybir
from concourse.bass import ds
from concourse.bass_utils import run_bass_kernel_spmd

AF = mybir.ActivationFunctionType
OP = mybir.AluOpType
F32 = mybir.dt.float32
BF16 = mybir.dt.bfloat16

B, T, N = 64, 512, 250
NP = 256          # n padded to 2 partition chunks
H = 512
G4 = 4 * H        # 2048
OUT = 1024
NCORES = 8
PB = B // NCORES  # 8 batches per core
WIN = 32          # scan steps per For_i iteration
NWIN = T // WIN

KC = H // 128     # 4 hidden chunks
GC = G4 // 128    # 16 gate chunks
NC2 = NP // 128   # 2 input chunks
CB = KC * PB      # 32 h/c columns


def _split_drain_waits(nc, max_waits=1):
    """This walrus build's CoreV3 codegen accepts at most one sem-wait per
    engine instruction (Drain/Matmult/... ISA structs have a single wait
    slot).  Move extra waits onto same-engine NoOps inserted immediately
    before the instruction — the engine blocks at the NoOp instead, which is
    semantically identical (same engine stream, same program point)."""
    for bb in nc.m.functions[0].blocks:
        insts = bb.instructions  # live list
        i = 0
        while i < len(insts):
            ins = insts[i]
            si = ins.sync_info
            if si is not None and len(si.on_wait) > max_waits:
                waits = list(si.on_wait)
                ins.sync_info = mybir.SyncInfo(
                    on_wait=waits[:max_waits], on_update=list(si.on_update)
                )
                for j, w in enumerate(waits[max_waits:]):
                    nop = mybir.InstNoOp(name=f"{ins.name}-wsplit{j}")
                    nop.engine = ins.engine
                    nop.sync_info = mybir.SyncInfo(on_wait=[w], on_update=[])
                    insts.insert(i, nop)
                    i += 1
            i += 1


def _emit_gates(nc, ps_pool, whh_sb, ident_sb, xw_u, st, tag):
    """PE part of one LSTM step: psum = xw_u (via identity matmul) + Whh @ h.

    The psum tile is padded to a full 2KB bank: matmul start=True clears
    has_written for the WHOLE bank, and Tile serializes it against readers
    of any other tile sharing the bank."""
    psum = ps_pool.tile([128, 512], F32, tag=f"ps{tag}", name=f"ps{tag}")
    nc.tensor.matmul(
        psum[:, 0:GC * PB].rearrange("p (g b) -> p g b", g=GC), ident_sb,
        xw_u, start=True, stop=False)
    for gc in range(GC):
        for kc in range(KC):
            nc.tensor.matmul(
                psum[:, gc * PB:(gc + 1) * PB],
                whh_sb[:, kc, gc * 128:(gc + 1) * 128],
                st["h_prev"][kc],
                start=False,
                stop=(gc == GC - 1 and kc == KC - 1),
            )
    st["psum"] = psum


def _emit_cell(nc, ew_pool, hw_pool, st, u, tag, order):
    """ACT/DVE part of one LSTM step (see module docstring).

    Gate column order is [g, i, f, o], so the sigmoids can start on the
    partial psum while the matmul burst is still running: g's columns are
    complete ~25% into the burst, i/f at ~75%; only o waits for the end.

    `order` holds the previous cell's tc (ACT) and h~ (DVE) instructions;
    ordering-only dep edges pin the engine FIFO order the Tile scheduler
    would otherwise break: this cell's sigmoids must come AFTER the
    previous cell's tc on ACT (else tc — which gates the next matmul burst
    via h~ — queues behind a sigmoid that waits for the burst's end), and
    this cell's first DVE op after the previous cell's h~."""
    psum = st["psum"]
    gc_cur = st["gc"][u % 2]
    gc_nxt = st["gc"][(u + 1) % 2]
    gsig = ew_pool.tile([128, CB], F32, tag=f"gs{tag}", name=f"gs{tag}")
    sig_i = nc.scalar.activation(gsig[:], psum[:, 0:CB], AF.Sigmoid)
    if order.get("tc") is not None:
        bass._add_dep_helper(sig_i.ins, order["tc"].ins, False, "ACT order")
    ts_i = nc.vector.tensor_scalar_add(gc_cur[:, 0:CB], gsig[:], -0.5)
    if order.get("h") is not None:
        bass._add_dep_helper(ts_i.ins, order["h"].ins, False, "DVE order")
    sif = ew_pool.tile([128, 2 * CB], F32, tag=f"if{tag}", name=f"if{tag}")
    nc.scalar.activation(sif[:], psum[:, CB:3 * CB], AF.Sigmoid)
    osig = ew_pool.tile([128, CB], F32, tag=f"os{tag}", name=f"os{tag}")
    nc.scalar.activation(osig[:], psum[:, 3 * CB:4 * CB], AF.Sigmoid)
    prods = ew_pool.tile([128, 2 * CB], F32, tag=f"pr{tag}", name=f"pr{tag}")
    nc.vector.tensor_tensor(prods[:], sif[:], gc_cur[:, 0:2 * CB], OP.mult)
    nc.vector.scalar_tensor_tensor(
        gc_nxt[:, CB:2 * CB], prods[:, 0:CB], 2.0, prods[:, CB:2 * CB],
        OP.mult, OP.add,
    )
    tc_t = ew_pool.tile([128, CB], F32, tag=f"tc{tag}", name=f"tc{tag}")
    tc_i = nc.scalar.activation(tc_t[:], gc_nxt[:, CB:2 * CB], AF.Sigmoid,
                                scale=2.0)
    acts_o = osig[:].rearrange("p (k b) -> p k b", k=KC)
    tc_v = tc_t[:].rearrange("p (k b) -> p k b", k=KC)
    if st["hwin_v"] is not None:
        h_out = st["hwin_v"][:, u]
    elif u == WIN - 1:
        h_out = st["h_cur"][:].rearrange("p (k b) -> p k b", k=KC)
    else:
        h_tmp = hw_pool.tile([128, CB], BF16, tag=f"h{tag}", name=f"h{tag}")
        h_out = h_tmp[:].rearrange("p (k b) -> p k b", k=KC)
    h_i = nc.vector.scalar_tensor_tensor(h_out, tc_v, -0.5, acts_o,
                                         OP.add, OP.mult)
    if st["hwin_v"] is not None and u == WIN - 1:
        nc.vector.tensor_copy(
            st["h_cur"][:].rearrange("p (k b) -> p k b", k=KC), h_out
        )
    st["h_prev"] = [h_out[:, kc, :] for kc in range(KC)]
    order["tc"] = tc_i
    order["h"] = h_i


def _emit_bulk_xw1(nc, bps_pool, a1_sb, b1_sb, hwin, stg):
    """stg = A1 @ h0(window) + b1, from the SBUF-resident h0 window
    (128, KC*WIN*PB) bf16 into the SBUF-persistent stg tile
    (128, GC*WIN*PB) bf16, cols = gc*(WIN*PB) + w*PB + b.  The next
    (sub-)window's L1 scan reads stg directly — no DRAM round-trip."""
    hv = hwin.rearrange("p (k wb) -> p k wb", k=KC)
    for gc in range(GC):
        psum = bps_pool.tile([128, 512], F32, tag="bps", name="bps")
        for kc in range(KC):
            nc.tensor.matmul(
                psum[:, 0:WIN * PB],
                a1_sb[:, kc, gc * 128:(gc + 1) * 128],
                hv[:, kc],
                start=(kc == 0),
                stop=(kc == KC - 1),
            )
        nc.scalar.activation(
            stg[:, gc * WIN * PB:(gc + 1) * WIN * PB], psum[:, 0:WIN * PB],
            AF.Identity, bias=b1_sb[:, gc:gc + 1],
        )


def build_nc():
    nc = bass.Bass()

    x_in = nc.dram_tensor("x", [PB, T, NP], BF16, kind="ExternalInput")
    gt_in = nc.dram_tensor("gt", [KC, 128, T], BF16, kind="ExternalInput")
    a0_in = nc.dram_tensor("a0t", [NC2, 128, G4], BF16, kind="ExternalInput")
    b0_in = nc.dram_tensor("b0", [128, GC], F32, kind="ExternalInput")
    whh0_in = nc.dram_tensor("whh0t", [KC, 128, G4], BF16, kind="ExternalInput")
    a1_in = nc.dram_tensor("a1t", [KC, 128, G4], BF16, kind="ExternalInput")
    b1_in = nc.dram_tensor("b1", [128, GC], F32, kind="ExternalInput")
    whh1_in = nc.dram_tensor("whh1t", [KC, 128, G4], BF16, kind="ExternalInput")
    wout_in = nc.dram_tensor("woutt", [KC, 128, OUT], BF16, kind="ExternalInput")
    bout_in = nc.dram_tensor("boutr", [PB, OUT], F32, kind="ExternalInput")
    ident_in = nc.dram_tensor("ident", [128, 128], BF16, kind="ExternalInput")
    out_ext = nc.dram_tensor("out", [PB, OUT], F32, kind="ExternalOutput")

    # (t,b)-interleaved for fast 1KB-segment window loads; one spare
    # window of scratch: pair-loads at the tail read one window past T.
    xw0_dram = nc.dram_tensor("xw0s", [GC, 128, (NWIN + 1) * WIN, PB], BF16)

    with tile.TileContext(nc) as tc:
        with ExitStack() as ctx:
            const_pool = ctx.enter_context(tc.tile_pool(name="consts", bufs=1))
            state_pool = ctx.enter_context(tc.tile_pool(name="state", bufs=1))

            gt_sb = const_pool.tile([128, KC, T], BF16)
            nc.sync.dma_start(gt_sb[:], gt_in[:, :, :].rearrange("k p t -> p k t"))
            a0_sb = const_pool.tile([128, NC2, G4], BF16)
            nc.sync.dma_start(a0_sb[:], a0_in[:, :, :].rearrange("k p g -> p k g"))
            b0_sb = const_pool.tile([128, GC], F32)
            nc.sync.dma_start(b0_sb[:], b0_in[:, :])
            whh0_sb = const_pool.tile([128, KC, G4], BF16)
            nc.sync.dma_start(whh0_sb[:], whh0_in[:, :, :].rearrange("k p g -> p k g"))
            a1_sb = const_pool.tile([128, KC, G4], BF16)
            nc.sync.dma_start(a1_sb[:], a1_in[:, :, :].rearrange("k p g -> p k g"))
            b1_sb = const_pool.tile([128, GC], F32)
            nc.sync.dma_start(b1_sb[:], b1_in[:, :])
            whh1_sb = const_pool.tile([128, KC, G4], BF16)
            nc.sync.dma_start(whh1_sb[:], whh1_in[:, :, :].rearrange("k p g -> p k g"))
            wout_sb = const_pool.tile([128, KC, OUT], BF16)
            nc.sync.dma_start(wout_sb[:], wout_in[:, :, :].rearrange("k p g -> p k g"))
            bout_sb = const_pool.tile([PB, OUT], F32)
            nc.sync.dma_start(bout_sb[:], bout_in[:, :])
            ident_sb = const_pool.tile([128, 128], BF16)
            nc.sync.dma_start(ident_sb[:], ident_in[:, :])

            # ---- phase A+B: featsT_b = x_bT @ G^T ; xw0 = A0 @ feats + b0 ----
            with tc.tile_pool(name="ab", bufs=3) as ab_pool, \
                 tc.tile_pool(name="abf", bufs=1) as abf_pool, \
                 tc.tile_pool(name="abps", bufs=4, space="PSUM") as abps_pool:
                feats = []
                for b in range(PB):
                    x_sb = ab_pool.tile([128, KC, NP], BF16, tag="x")
                    nc.sync.dma_start(
                        x_sb[:], x_in[b].rearrange("(k p) n -> p k n", p=128)
                    )
                    fb = abf_pool.tile([128, NC2, T], BF16, tag=f"feats{b}")
                    for mc in range(NC2):
                        psA = abps_pool.tile([128, T], F32, tag="psA")
                        for kc in range(KC):
                            nc.tensor.matmul(
                                psA[:],
                                x_sb[:, kc, mc * 128:(mc + 1) * 128],
                                gt_sb[:, kc, :],
                                start=(kc == 0),
                                stop=(kc == KC - 1),
                            )
                        nc.vector.tensor_copy(fb[:, mc, :], psA[:])
                    feats.append(fb)
                TC8 = T // 64  # 8 chunks of 64 timesteps
                for gc in range(GC):
                    # produce xw0 (t,b)-interleaved: strided psum matmul
                    # writes (per-b column combs), contiguous ACT reads.
                    xw_sb = ab_pool.tile([128, T * PB], BF16, tag="xw")
                    for t8 in range(TC8):
                        psB = abps_pool.tile([128, 512], F32, tag="psB")
                        psv = psB[:].rearrange("p (w b) -> p w b", b=PB)
                        for b in range(PB):
                            for kc in range(NC2):
                                nc.tensor.matmul(
                                    psv[:, :, b],
                                    a0_sb[:, kc, gc * 128:(gc + 1) * 128],
                                    feats[b][:, kc, t8 * 64:(t8 + 1) * 64],
                                    start=(b == 0 and kc == 0),
                                    stop=(b == PB - 1 and kc == NC2 - 1),
                                )
                        nc.scalar.activation(
                            xw_sb[:, t8 * 512:(t8 + 1) * 512], psB[:],
                            AF.Identity, bias=b0_sb[:, gc:gc + 1],
                        )
                    nc.sync.dma_start(
                        xw0_dram[gc, :, 0:T, :].rearrange("p t b -> p (t b)"),
                        xw_sb[:],
                    )

            # ---- merged scans: L0 windows 0..15, L1 lagging one window ----
            h0_cur = state_pool.tile([128, CB], BF16)
            h1_cur = state_pool.tile([128, CB], BF16)
            gc0 = [state_pool.tile([128, 2 * CB], F32, name=f"gc0_{i}")
                   for i in range(2)]
            gc1 = [state_pool.tile([128, 2 * CB], F32, name=f"gc1_{i}")
                   for i in range(2)]
            for t_ in (h0_cur, h1_cur):
                nc.vector.memset(t_[:], 0.0)
            for t_ in (*gc0, *gc1):
                nc.vector.memset(t_[:], 0.0)

            st0 = {"h_cur": h0_cur, "gc": gc0}
            st1 = {"h_cur": h1_cur, "gc": gc1, "hwin_v": None}

            with ExitStack() as sctx:
                win_pool = sctx.enter_context(tc.tile_pool(name="win", bufs=2))
                ps0_pool = sctx.enter_context(
                    tc.tile_pool(name="ps0", bufs=2, space="PSUM"))
                ps1_pool = sctx.enter_context(
                    tc.tile_pool(name="ps1", bufs=2, space="PSUM"))
                bps_pool = sctx.enter_context(
                    tc.tile_pool(name="bps", bufs=3, space="PSUM"))
                ew_pool = sctx.enter_context(tc.tile_pool(name="ew", bufs=3))
                hw_pool = sctx.enter_context(tc.tile_pool(name="hw", bufs=3))

                # xw1 stays SBUF-resident: bulk for window w writes one of
                # these; the L1 scan of window w reads it next (sub-)window.
                stgA = state_pool.tile([128, GC * WIN * PB], BF16)
                stgB = state_pool.tile([128, GC * WIN * PB], BF16)

                def l0_load_pair(wi):
                    """Load xw0 windows wi and wi+1 in ONE dynamic DMA (two
                    separate dynamic-offset loads per loop body break AP
                    lowering); (w b) source is contiguous -> 1KB segments."""
                    win0 = win_pool.tile([128, GC * 2 * WIN * PB], BF16,
                                         tag="w0", name="w0")
                    nc.sync.dma_start(
                        win0[:].rearrange(
                            "p (g w b) -> p g w b", g=GC, w=2 * WIN),
                        xw0_dram[:, :, ds(wi * WIN, 2 * WIN), :].rearrange(
                            "g p w b -> p g w b"),
                    )
                    return win0

                def l0_window(win0, half):
                    hwin = win_pool.tile([128, KC * WIN * PB], BF16, tag="hw0",
                                         name="hw0")
                    st0["hwin_v"] = hwin.rearrange(
                        "p (k w b) -> p w k b", k=KC, w=WIN)
                    win0_v = win0[:].rearrange(
                        "p (g w b) -> p w g b", g=GC, w=2 * WIN)[
                        :, half * WIN:(half + 1) * WIN]
                    st0["h_prev"] = [
                        h0_cur[:, kc * PB:(kc + 1) * PB] for kc in range(KC)]
                    return win0_v, hwin

                def l1_window(stg):
                    win1_v = stg[:].rearrange(
                        "p (g w b) -> p w g b", g=GC, w=WIN)
                    st1["h_prev"] = [
                        h1_cur[:, kc * PB:(kc + 1) * PB] for kc in range(KC)]
                    return win1_v

                def sub_body(win0, half, stg_in, stg_out, order):
                    """One window: L0 scan of window half `half` of the
                    pair-loaded win0 tile, L1 scan one window behind
                    reading stg_in, bulk xw1 for this L0 window into
                    stg_out."""
                    win0_v, hwin = l0_window(win0, half)
                    win1_v = l1_window(stg_in)
                    for u in range(WIN):
                        _emit_gates(nc, ps0_pool, whh0_sb, ident_sb,
                                    win0_v[:, u], st0, "0")
                        _emit_gates(nc, ps1_pool, whh1_sb, ident_sb,
                                    win1_v[:, u], st1, "1")
                        _emit_cell(nc, ew_pool, hw_pool, st0, u, "0", order)
                        _emit_cell(nc, ew_pool, hw_pool, st1, u, "1", order)
                    _emit_bulk_xw1(nc, bps_pool, a1_sb, b1_sb, hwin, stg_out)

                # peel: L0 window 0 alone, then bulk xw1 window 0 -> stgA
                order = {}
                win0_v, hwin = l0_window(l0_load_pair(0), 0)
                for u in range(WIN):
                    _emit_gates(nc, ps0_pool, whh0_sb, ident_sb,
                                win0_v[:, u], st0, "0")
                    _emit_cell(nc, ew_pool, hw_pool, st0, u, "0", order)
                _emit_bulk_xw1(nc, bps_pool, a1_sb, b1_sb, hwin, stgA)

                # main loop, unrolled 2x: sub A handles L0 w(2j+1)/L1 w(2j)
                # (xw1 from stgA, bulk -> stgB); sub B handles the next pair
                # with stg roles swapped.  7 iterations cover L0 w1..w14.
                with tc.For_i(0, NWIN // 2 - 1, 1,
                              hint_engines=(mybir.EngineType.PE,
                                            mybir.EngineType.Activation,
                                            mybir.EngineType.DVE,
                                            mybir.EngineType.SP,
                                            mybir.EngineType.Pool),
                              staggered_reset=True) as jw:
                    order = {}  # no cross-basic-block ordering edges
                    win0 = l0_load_pair(jw * 2 + 1)
                    sub_body(win0, 0, stgA, stgB, order)
                    sub_body(win0, 1, stgB, stgA, order)

                # post: L0 w15 with L1 w14 (stgA), bulk w15 -> stgB
                order = {}
                sub_body(l0_load_pair(NWIN - 1), 0, stgA, stgB, order)

                # peel: L1 last window (w15) from stgB
                order = {}
                win1_v = l1_window(stgB)
                for u in range(WIN):
                    _emit_gates(nc, ps1_pool, whh1_sb, ident_sb,
                                win1_v[:, u], st1, "1")
                    _emit_cell(nc, ew_pool, hw_pool, st1, u, "1", order)

            # ---- phase F: out = relu(h1_last @ Wout.T + bout) ----
            with tc.tile_pool(name="f_ps", bufs=2, space="PSUM") as fps_pool, \
                 tc.tile_pool(name="f_o", bufs=1) as fo_pool:
                out_sb = fo_pool.tile([PB, OUT], F32)
                for half in range(2):
                    psF = fps_pool.tile([PB, 512], F32, tag="psF")
                    for kc in range(KC):
                        nc.tensor.matmul(
                            psF[:],
                            h1_cur[:, kc * PB:(kc + 1) * PB],
                            wout_sb[:, kc, half * 512:(half + 1) * 512],
                            start=(kc == 0),
                            stop=(kc == KC - 1),
                        )
                    sl = slice(half * 512, (half + 1) * 512)
                    nc.vector.tensor_tensor(
                        out_sb[:, sl], psF[:], bout_sb[:, sl], OP.add
                    )
                    nc.vector.tensor_scalar_max(out_sb[:, sl], out_sb[:, sl], 0.0)
                nc.sync.dma_start(out_ext[:, :], out_sb[:])

    _split_drain_waits(nc)
    return nc


_NC_CACHE = None


def _get_nc():
    global _NC_CACHE
    if _NC_CACHE is None:
        _NC_CACHE = build_nc()
    return _NC_CACHE


def _prep_host(inputs):
    x = np.asarray(inputs["x"], dtype=np.float32)
    coef = 1.0 / math.gamma(0.5)
    t = np.arange(T, dtype=np.float64)
    diff = t[:, None] - t[None, :]
    W = np.where(diff > 0, (np.abs(diff) + 1e-6) ** -0.5, 0.0).astype(np.float32)
    d = (coef * W.sum(1)).astype(np.float32)
    G = (np.diag(d) - coef * W).astype(np.float32)  # feats_b = G @ x_b
    GT = np.ascontiguousarray(G.T).astype(ml_dtypes.bfloat16).reshape(KC, 128, T)

    perm = np.concatenate([  # torch gate order i,f,g,o -> [g,i,f,o]
        np.arange(2 * H, 3 * H), np.arange(0, H),
        np.arange(H, 2 * H), np.arange(3 * H, 4 * H),
    ])
    bf = ml_dtypes.bfloat16
    # tanh-as-sigmoid folding: g-gate rows x2; h~=h/2 so h-consumers x2.
    gsc = np.ones((G4, 1), np.float32)
    gsc[0:H] = 2.0

    A0 = np.zeros((G4, NP), np.float32)
    A0[:, :N] = np.asarray(inputs["Wih0"], np.float32)[perm, :N] * gsc
    A0T = np.ascontiguousarray(A0.T).astype(bf).reshape(NC2, 128, G4)
    b0 = ((np.asarray(inputs["bih0"], np.float32)
           + np.asarray(inputs["bhh0"], np.float32))[perm] * gsc[:, 0])
    b0_t = np.ascontiguousarray(b0.reshape(GC, 128).T)
    Whh0T = np.ascontiguousarray(
        (np.asarray(inputs["Whh0"], np.float32)[perm] * gsc * 2.0).T
    ).astype(bf).reshape(KC, 128, G4)

    A1T = np.ascontiguousarray(
        (np.asarray(inputs["Wih1"], np.float32)[perm] * gsc * 2.0).T
    ).astype(bf).reshape(KC, 128, G4)
    b1 = ((np.asarray(inputs["bih1"], np.float32)
           + np.asarray(inputs["bhh1"], np.float32))[perm] * gsc[:, 0])
    b1_t = np.ascontiguousarray(b1.reshape(GC, 128).T)
    Whh1T = np.ascontiguousarray(
        (np.asarray(inputs["Whh1"], np.float32)[perm] * gsc * 2.0).T
    ).astype(bf).reshape(KC, 128, G4)

    WoutT = np.ascontiguousarray(
        (np.asarray(inputs["Wout"], np.float32) * 2.0).T
    ).astype(bf).reshape(KC, 128, OUT)
    bout_r = np.broadcast_to(
        np.asarray(inputs["bout"], np.float32), (PB, OUT)
    ).copy()

    ident = np.eye(128, dtype=np.float32).astype(bf)

    xp = np.zeros((B, T, NP), ml_dtypes.bfloat16)
    xp[:, :, :N] = x.astype(ml_dtypes.bfloat16)

    shared = dict(
        gt=GT, a0t=A0T, b0=b0_t, whh0t=Whh0T, a1t=A1T, b1=b1_t,
        whh1t=Whh1T, woutt=WoutT, boutr=bout_r, ident=ident,
    )
    in_maps = []
    for c in range(NCORES):
        m = dict(shared)
        m["x"] = np.ascontiguousarray(xp[c * PB:(c + 1) * PB])
        in_maps.append(m)
    return in_maps


def kernel(**inputs):
    nc = _get_nc()
    in_maps = _prep_host(inputs)
    res = run_bass_kernel_spmd(nc, in_maps, core_ids=list(range(NCORES)))
    out = np.concatenate([r["out"] for r in res.results], axis=0)
    return out.astype(np.float32)


# revision 33
# speedup vs baseline: 1.1705x; 1.1705x over previous
"""Trainium2 Bass kernel for the CaputoEncoder model.

Model (see reference): feats = concat([caputo(x, 0.5), caputo(x, 1.0)], -1)
-> 2-layer LSTM(512) -> last timestep -> relu(linear).

Key simplifications:
  * caputo(x, 1.0) has coefficient 1/gamma(0) == 0 -> contributes zeros;
    only the alpha=0.5 branch matters, so only Wih0[:, :250] is ever used.
  * caputo(x, .5) = d*x - Wc@x (over time) == G @ x_b with G = diag(d) - Wc,
    host-precomputed; becomes a single matmul per batch.

Sharding: pure data parallelism over batch (64 -> 8 per core, 8 cores).
All weights replicated; scatter/gather on host.

Layer pipelining: the two LSTM scans run interleaved step-by-step, layer 1
one window (32 steps) behind layer 0, in a 2x-unrolled hardware loop.  Per
step the PE runs 1 identity-matmul (injects xw_t into PSUM) + 64 Whh
matmuls per layer; emission order is gates(L0), gates(L1), cell(L0),
cell(L1) so each layer's cell chain hides under the other layer's matmul
burst.  xw1 = A1 @ h0-window stays SBUF-resident (stgA/stgB ping-pong, no
DRAM round-trip); xw0 windows are pair-loaded per loop body (a single
dynamic-offset DMA group per body — two break AP lowering).

Cell chain (per layer, 3+1 ACT + 4 DVE ops): with tanh(x) = 2*sigmoid(2x)-1
and h~ := h/2 stored everywhere (consumer weights pre-doubled host-side,
g-gate rows pre-scaled 2x):
    g/i,f/o = sigmoid(psum cols)          # ACT, split so g starts ~25% into
                                          # the burst, i/f at ~75% (partial-
                                          # psum region deps)
    g~   = sig_g - 0.5                    # DVE tensor_scalar
    prods= [i|f] * [g~|c_prev]            # DVE, one 64-col multiply
    c    = 2*prods_ig + prods_fc          # DVE scalar_tensor_tensor
    tc   = sigmoid(2*c)                   # ACT
    h~   = (tc - 0.5) * o                 # DVE scalar_tensor_tensor
Ordering-only dep edges (tc before next sigmoid on ACT, h~ before next
tensor_scalar on DVE) pin the engine FIFO order the Tile scheduler would
otherwise break.  Full-bank psum tiles prevent start=True has_written
clears from serializing against the other pool's readers.

On-core layout (hidden-major):
  h~, c   : (128 part = hidden%128, cols = kchunk*8 + b)   [4*8=32 cols]
  gatesT  : (128 part = gate%128,  cols = gchunk*8 + b)    [16*8=128 cols]
  gate chunks host-permuted to [g, i, f, o].
"""

import math
from contextlib import ExitStack

import numpy as np
import ml_dtypes

import concourse.bass as bass
import concourse.tile as tile
from concourse import mybir
from concourse.bass import ds
from concourse.bass_utils import run_bass_kernel_spmd

AF = mybir.ActivationFunctionType
OP = mybir.AluOpType
F32 = mybir.dt.float32
BF16 = mybir.dt.bfloat16

B, T, N = 64, 512, 250
NP = 256          # n padded to 2 partition chunks
H = 512
G4 = 4 * H        # 2048
OUT = 1024
NCORES = 8
PB = B // NCORES  # 8 batches per core
WIN = 32          # scan steps per For_i iteration
NWIN = T // WIN

KC = H // 128     # 4 hidden chunks
GC = G4 // 128    # 16 gate chunks
NC2 = NP // 128   # 2 input chunks
CB = KC * PB      # 32 h/c columns


def _split_drain_waits(nc, max_waits=1):
    """This walrus build's CoreV3 codegen accepts at most one sem-wait per
    engine instruction (Drain/Matmult/... ISA structs have a single wait
    slot).  Move extra waits onto same-engine NoOps inserted immediately
    before the instruction — the engine blocks at the NoOp instead, which is
    semantically identical (same engine stream, same program point)."""
    for bb in nc.m.functions[0].blocks:
        insts = bb.instructions  # live list
        i = 0
        while i < len(insts):
            ins = insts[i]
            si = ins.sync_info
            if si is not None and len(si.on_wait) > max_waits:
                waits = list(si.on_wait)
                ins.sync_info = mybir.SyncInfo(
                    on_wait=waits[:max_waits], on_update=list(si.on_update)
                )
                for j, w in enumerate(waits[max_waits:]):
                    nop = mybir.InstNoOp(name=f"{ins.name}-wsplit{j}")
                    nop.engine = ins.engine
                    nop.sync_info = mybir.SyncInfo(on_wait=[w], on_update=[])
                    insts.insert(i, nop)
                    i += 1
            i += 1


def _emit_gates(nc, ps_pool, whh_sb, ident_sb, xw_u, st, tag):
    """PE part of one LSTM step: psum = xw_u (via identity matmul) + Whh @ h.

    The psum tile is padded to a full 2KB bank: matmul start=True clears
    has_written for the WHOLE bank, and Tile serializes it against readers
    of any other tile sharing the bank."""
    psum = ps_pool.tile([128, 512], F32, tag=f"ps{tag}", name=f"ps{tag}")
    nc.tensor.matmul(
        psum[:, 0:GC * PB].rearrange("p (g b) -> p g b", g=GC), ident_sb,
        xw_u, start=True, stop=False)
    for gc in range(GC):
        for kc in range(KC):
            nc.tensor.matmul(
                psum[:, gc * PB:(gc + 1) * PB],
                whh_sb[:, kc, gc * 128:(gc + 1) * 128],
                st["h_prev"][kc],
                start=False,
                stop=(gc == GC - 1 and kc == KC - 1),
            )
    st["psum"] = psum


def _emit_cell(nc, ew_pool, hw_pool, st, u, tag, order):
    """ACT/DVE part of one LSTM step (see module docstring).

    Gate column order is [g, i, f, o], so the sigmoids can start on the
    partial psum while the matmul burst is still running: g's columns are
    complete ~25% into the burst, i/f at ~75%; only o waits for the end.

    `order` holds the previous cell's tc (ACT) and h~ (DVE) instructions;
    ordering-only dep edges pin the engine FIFO order the Tile scheduler
    would otherwise break: this cell's sigmoids must come AFTER the
    previous cell's tc on ACT (else tc — which gates the next matmul burst
    via h~ — queues behind a sigmoid that waits for the burst's end), and
    this cell's first DVE op after the previous cell's h~."""
    psum = st["psum"]
    gc_cur = st["gc"][u % 2]
    gc_nxt = st["gc"][(u + 1) % 2]
    gsig = ew_pool.tile([128, CB], F32, tag=f"gs{tag}", name=f"gs{tag}")
    sig_i = nc.scalar.activation(gsig[:], psum[:, 0:CB], AF.Sigmoid)
    if order.get("tc") is not None:
        bass._add_dep_helper(sig_i.ins, order["tc"].ins, False, "ACT order")
    ts_i = nc.vector.tensor_scalar_add(gc_cur[:, 0:CB], gsig[:], -0.5)
    if order.get("h") is not None:
        bass._add_dep_helper(ts_i.ins, order["h"].ins, False, "DVE order")
    sif = ew_pool.tile([128, 2 * CB], F32, tag=f"if{tag}", name=f"if{tag}")
    nc.scalar.activation(sif[:], psum[:, CB:3 * CB], AF.Sigmoid)
    osig = ew_pool.tile([128, CB], F32, tag=f"os{tag}", name=f"os{tag}")
    nc.scalar.activation(osig[:], psum[:, 3 * CB:4 * CB], AF.Sigmoid)
    prods = ew_pool.tile([128, 2 * CB], F32, tag=f"pr{tag}", name=f"pr{tag}")
    nc.vector.tensor_tensor(prods[:], sif[:], gc_cur[:, 0:2 * CB], OP.mult)
    nc.vector.scalar_tensor_tensor(
        gc_nxt[:, CB:2 * CB], prods[:, 0:CB], 2.0, prods[:, CB:2 * CB],
        OP.mult, OP.add,
    )
    tc_t = ew_pool.tile([128, CB], F32, tag=f"tc{tag}", name=f"tc{tag}")
    tc_i = nc.scalar.activation(tc_t[:], gc_nxt[:, CB:2 * CB], AF.Sigmoid,
                                scale=2.0)
    acts_o = osig[:].rearrange("p (k b) -> p k b", k=KC)
    tc_v = tc_t[:].rearrange("p (k b) -> p k b", k=KC)
    if st["hwin_v"] is not None:
        h_out = st["hwin_v"][:, u]
    elif u == WIN - 1:
        h_out = st["h_cur"][:].rearrange("p (k b) -> p k b", k=KC)
    else:
        h_tmp = hw_pool.tile([128, CB], BF16, tag=f"h{tag}", name=f"h{tag}")
        h_out = h_tmp[:].rearrange("p (k b) -> p k b", k=KC)
    h_i = nc.vector.scalar_tensor_tensor(h_out, tc_v, -0.5, acts_o,
                                         OP.add, OP.mult)
    if st["hwin_v"] is not None and u == WIN - 1:
        nc.vector.tensor_copy(
            st["h_cur"][:].rearrange("p (k b) -> p k b", k=KC), h_out
        )
    st["h_prev"] = [h_out[:, kc, :] for kc in range(KC)]
    order["tc"] = tc_i
    order["h"] = h_i


def _emit_bulk_xw1(nc, bps_pool, a1_sb, b1_sb, hwin, stg):
    """stg = A1 @ h0(window) + b1, from the SBUF-resident h0 window
    (128, KC*WIN*PB) bf16 into the SBUF-persistent stg tile
    (128, GC*WIN*PB) bf16, cols = gc*(WIN*PB) + w*PB + b.  The next
    (sub-)window's L1 scan reads stg directly — no DRAM round-trip."""
    hv = hwin.rearrange("p (k wb) -> p k wb", k=KC)
    for gc in range(GC):
        psum = bps_pool.tile([128, 512], F32, tag="bps", name="bps")
        for kc in range(KC):
            nc.tensor.matmul(
                psum[:, 0:WIN * PB],
                a1_sb[:, kc, gc * 128:(gc + 1) * 128],
                hv[:, kc],
                start=(kc == 0),
                stop=(kc == KC - 1),
            )
        nc.scalar.activation(
            stg[:, gc * WIN * PB:(gc + 1) * WIN * PB], psum[:, 0:WIN * PB],
            AF.Identity, bias=b1_sb[:, gc:gc + 1],
        )


def build_nc():
    nc = bass.Bass()

    x_in = nc.dram_tensor("x", [PB, T, NP], BF16, kind="ExternalInput")
    gt_in = nc.dram_tensor("gt", [KC, 128, T], BF16, kind="ExternalInput")
    a0_in = nc.dram_tensor("a0t", [NC2, 128, G4], BF16, kind="ExternalInput")
    b0_in = nc.dram_tensor("b0", [128, GC], F32, kind="ExternalInput")
    whh0_in = nc.dram_tensor("whh0t", [KC, 128, G4], BF16, kind="ExternalInput")
    a1_in = nc.dram_tensor("a1t", [KC, 128, G4], BF16, kind="ExternalInput")
    b1_in = nc.dram_tensor("b1", [128, GC], F32, kind="ExternalInput")
    whh1_in = nc.dram_tensor("whh1t", [KC, 128, G4], BF16, kind="ExternalInput")
    wout_in = nc.dram_tensor("woutt", [KC, 128, OUT], BF16, kind="ExternalInput")
    bout_in = nc.dram_tensor("boutr", [PB, OUT], F32, kind="ExternalInput")
    ident_in = nc.dram_tensor("ident", [128, 128], BF16, kind="ExternalInput")
    out_ext = nc.dram_tensor("out", [PB, OUT], F32, kind="ExternalOutput")

    # (t,b)-interleaved for fast 1KB-segment window loads; one spare
    # window of scratch: pair-loads at the tail read one window past T.
    xw0_dram = nc.dram_tensor("xw0s", [GC, 128, (NWIN + 1) * WIN, PB], BF16)

    with tile.TileContext(nc) as tc:
        with ExitStack() as ctx:
            const_pool = ctx.enter_context(tc.tile_pool(name="consts", bufs=1))
            state_pool = ctx.enter_context(tc.tile_pool(name="state", bufs=1))

            gt_sb = const_pool.tile([128, KC, T], BF16)
            nc.sync.dma_start(gt_sb[:], gt_in[:, :, :].rearrange("k p t -> p k t"))
            a0_sb = const_pool.tile([128, NC2, G4], BF16)
            nc.sync.dma_start(a0_sb[:], a0_in[:, :, :].rearrange("k p g -> p k g"))
            b0_sb = const_pool.tile([128, GC], F32)
            nc.sync.dma_start(b0_sb[:], b0_in[:, :])
            whh0_sb = const_pool.tile([128, KC, G4], BF16)
            nc.sync.dma_start(whh0_sb[:], whh0_in[:, :, :].rearrange("k p g -> p k g"))
            a1_sb = const_pool.tile([128, KC, G4], BF16)
            nc.sync.dma_start(a1_sb[:], a1_in[:, :, :].rearrange("k p g -> p k g"))
            b1_sb = const_pool.tile([128, GC], F32)
            nc.sync.dma_start(b1_sb[:], b1_in[:, :])
            whh1_sb = const_pool.tile([128, KC, G4], BF16)
            nc.sync.dma_start(whh1_sb[:], whh1_in[:, :, :].rearrange("k p g -> p k g"))
            wout_sb = const_pool.tile([128, KC, OUT], BF16)
            nc.sync.dma_start(wout_sb[:], wout_in[:, :, :].rearrange("k p g -> p k g"))
            bout_sb = const_pool.tile([PB, OUT], F32)
            nc.sync.dma_start(bout_sb[:], bout_in[:, :])
            ident_sb = const_pool.tile([128, 128], BF16)
            nc.sync.dma_start(ident_sb[:], ident_in[:, :])

            # ---- phase A+B: featsT_b = x_bT @ G^T ; xw0 = A0 @ feats + b0 ----
            with tc.tile_pool(name="ab", bufs=3) as ab_pool, \
                 tc.tile_pool(name="abf", bufs=1) as abf_pool, \
                 tc.tile_pool(name="abps", bufs=4, space="PSUM") as abps_pool:
                feats = []
                for b in range(PB):
                    x_sb = ab_pool.tile([128, KC, NP], BF16, tag="x")
                    nc.sync.dma_start(
                        x_sb[:], x_in[b].rearrange("(k p) n -> p k n", p=128)
                    )
                    fb = abf_pool.tile([128, NC2, T], BF16, tag=f"feats{b}")
                    for mc in range(NC2):
                        psA = abps_pool.tile([128, T], F32, tag="psA")
                        for kc in range(KC):
                            nc.tensor.matmul(
                                psA[:],
                                x_sb[:, kc, mc * 128:(mc + 1) * 128],
                                gt_sb[:, kc, :],
                                start=(kc == 0),
                                stop=(kc == KC - 1),
                            )
                        nc.vector.tensor_copy(fb[:, mc, :], psA[:])
                    feats.append(fb)
                TC8 = T // 64  # 8 chunks of 64 timesteps
                for gc in range(GC):
                    # produce xw0 (t,b)-interleaved: strided psum matmul
                    # writes (per-b column combs), contiguous ACT reads.
                    xw_sb = ab_pool.tile([128, T * PB], BF16, tag="xw")
                    for t8 in range(TC8):
                        psB = abps_pool.tile([128, 512], F32, tag="psB")
                        psv = psB[:].rearrange("p (w b) -> p w b", b=PB)
                        for b in range(PB):
                            for kc in range(NC2):
                                nc.tensor.matmul(
                                    psv[:, :, b],
                                    a0_sb[:, kc, gc * 128:(gc + 1) * 128],
                                    feats[b][:, kc, t8 * 64:(t8 + 1) * 64],
                                    start=(b == 0 and kc == 0),
                                    stop=(b == PB - 1 and kc == NC2 - 1),
                                )
                        nc.scalar.activation(
                            xw_sb[:, t8 * 512:(t8 + 1) * 512], psB[:],
                            AF.Identity, bias=b0_sb[:, gc:gc + 1],
                        )
                    nc.sync.dma_start(
                        xw0_dram[gc, :, 0:T, :].rearrange("p t b -> p (t b)"),
                        xw_sb[:],
                    )

            # ---- merged scans: L0 windows 0..15, L1 lagging one window ----
            h0_cur = state_pool.tile([128, CB], BF16)
            h1_cur = state_pool.tile([128, CB], BF16)
            gc0 = [state_pool.tile([128, 2 * CB], F32, name=f"gc0_{i}")
                   for i in range(2)]
            gc1 = [state_pool.tile([128, 2 * CB], F32, name=f"gc1_{i}")
                   for i in range(2)]
            for t_ in (h0_cur, h1_cur):
                nc.vector.memset(t_[:], 0.0)
            for t_ in (*gc0, *gc1):
                nc.vector.memset(t_[:], 0.0)

            st0 = {"h_cur": h0_cur, "gc": gc0}
            st1 = {"h_cur": h1_cur, "gc": gc1, "hwin_v": None}

            with ExitStack() as sctx:
                win_pool = sctx.enter_context(tc.tile_pool(name="win", bufs=2))
                ps0_pool = sctx.enter_context(
                    tc.tile_pool(name="ps0", bufs=2, space="PSUM"))
                ps1_pool = sctx.enter_context(
                    tc.tile_pool(name="ps1", bufs=2, space="PSUM"))
                bps_pool = sctx.enter_context(
                    tc.tile_pool(name="bps", bufs=2, space="PSUM"))
                ew_pool = sctx.enter_context(tc.tile_pool(name="ew", bufs=3))
                hw_pool = sctx.enter_context(tc.tile_pool(name="hw", bufs=3))

                # xw1 stays SBUF-resident: bulk for window w writes one of
                # these; the L1 scan of window w reads it next (sub-)window.
                stgA = state_pool.tile([128, GC * WIN * PB], BF16)
                stgB = state_pool.tile([128, GC * WIN * PB], BF16)

                def l0_load_pair(wi):
                    """Load xw0 windows wi and wi+1 in ONE dynamic DMA (two
                    separate dynamic-offset loads per loop body break AP
                    lowering); (w b) source is contiguous -> 1KB segments."""
                    win0 = win_pool.tile([128, GC * 2 * WIN * PB], BF16,
                                         tag="w0", name="w0")
                    nc.sync.dma_start(
                        win0[:].rearrange(
                            "p (g w b) -> p g w b", g=GC, w=2 * WIN),
                        xw0_dram[:, :, ds(wi * WIN, 2 * WIN), :].rearrange(
                            "g p w b -> p g w b"),
                    )
                    return win0

                def l0_window(win0, half):
                    hwin = win_pool.tile([128, KC * WIN * PB], BF16, tag="hw0",
                                         name="hw0")
                    st0["hwin_v"] = hwin.rearrange(
                        "p (k w b) -> p w k b", k=KC, w=WIN)
                    win0_v = win0[:].rearrange(
                        "p (g w b) -> p w g b", g=GC, w=2 * WIN)[
                        :, half * WIN:(half + 1) * WIN]
                    st0["h_prev"] = [
                        h0_cur[:, kc * PB:(kc + 1) * PB] for kc in range(KC)]
                    return win0_v, hwin

                def l1_window(stg):
                    win1_v = stg[:].rearrange(
                        "p (g w b) -> p w g b", g=GC, w=WIN)
                    st1["h_prev"] = [
                        h1_cur[:, kc * PB:(kc + 1) * PB] for kc in range(KC)]
                    return win1_v

                def sub_body(win0, half, stg_in, stg_out, order):
                    """One window: L0 scan of window half `half` of the
                    pair-loaded win0 tile, L1 scan one window behind
                    reading stg_in, bulk xw1 for this L0 window into
                    stg_out."""
                    win0_v, hwin = l0_window(win0, half)
                    win1_v = l1_window(stg_in)
                    for u in range(WIN):
                        _emit_gates(nc, ps0_pool, whh0_sb, ident_sb,
                                    win0_v[:, u], st0, "0")
                        _emit_gates(nc, ps1_pool, whh1_sb, ident_sb,
                                    win1_v[:, u], st1, "1")
                        _emit_cell(nc, ew_pool, hw_pool, st0, u, "0", order)
                        _emit_cell(nc, ew_pool, hw_pool, st1, u, "1", order)
                    _emit_bulk_xw1(nc, bps_pool, a1_sb, b1_sb, hwin, stg_out)

                # peel: L0 window 0 alone, then bulk xw1 window 0 -> stgA
                order = {}
                win0_v, hwin = l0_window(l0_load_pair(0), 0)
                for u in range(WIN):
                    _emit_gates(nc, ps0_pool, whh0_sb, ident_sb,
                                win0_v[:, u], st0, "0")
                    _emit_cell(nc, ew_pool, hw_pool, st0, u, "0", order)
                _emit_bulk_xw1(nc, bps_pool, a1_sb, b1_sb, hwin, stgA)

                # main loop, unrolled 2x: sub A handles L0 w(2j+1)/L1 w(2j)
                # (xw1 from stgA, bulk -> stgB); sub B handles the next pair
                # with stg roles swapped.  7 iterations cover L0 w1..w14.
                with tc.For_i(0, NWIN // 2 - 1, 1,
                              hint_engines=(mybir.EngineType.PE,
                                            mybir.EngineType.Activation,
                                            mybir.EngineType.DVE,
                                            mybir.EngineType.SP,
                                            mybir.EngineType.Pool),
                              staggered_reset=True) as jw:
                    order = {}  # no cross-basic-block ordering edges
                    win0 = l0_load_pair(jw * 2 + 1)
                    sub_body(win0, 0, stgA, stgB, order)
                    sub_body(win0, 1, stgB, stgA, order)

                # post: L0 w15 with L1 w14 (stgA), bulk w15 -> stgB
                order = {}
                sub_body(l0_load_pair(NWIN - 1), 0, stgA, stgB, order)

                # peel: L1 last window (w15) from stgB
                order = {}
                win1_v = l1_window(stgB)
                for u in range(WIN):
                    _emit_gates(nc, ps1_pool, whh1_sb, ident_sb,
                                win1_v[:, u], st1, "1")
                    _emit_cell(nc, ew_pool, hw_pool, st1, u, "1", order)

            # ---- phase F: out = relu(h1_last @ Wout.T + bout) ----
            with tc.tile_pool(name="f_ps", bufs=2, space="PSUM") as fps_pool, \
                 tc.tile_pool(name="f_o", bufs=1) as fo_pool:
                out_sb = fo_pool.tile([PB, OUT], F32)
                for half in range(2):
                    psF = fps_pool.tile([PB, 512], F32, tag="psF")
                    for kc in range(KC):
                        nc.tensor.matmul(
                            psF[:],
                            h1_cur[:, kc * PB:(kc + 1) * PB],
                            wout_sb[:, kc, half * 512:(half + 1) * 512],
                            start=(kc == 0),
                            stop=(kc == KC - 1),
                        )
                    sl = slice(half * 512, (half + 1) * 512)
                    nc.vector.tensor_tensor(
                        out_sb[:, sl], psF[:], bout_sb[:, sl], OP.add
                    )
                    nc.vector.tensor_scalar_max(out_sb[:, sl], out_sb[:, sl], 0.0)
                nc.sync.dma_start(out_ext[:, :], out_sb[:])

    _split_drain_waits(nc)
    return nc


_NC_CACHE = None


def _get_nc():
    global _NC_CACHE
    if _NC_CACHE is None:
        _NC_CACHE = build_nc()
    return _NC_CACHE


def _prep_host(inputs):
    x = np.asarray(inputs["x"], dtype=np.float32)
    coef = 1.0 / math.gamma(0.5)
    t = np.arange(T, dtype=np.float64)
    diff = t[:, None] - t[None, :]
    W = np.where(diff > 0, (np.abs(diff) + 1e-6) ** -0.5, 0.0).astype(np.float32)
    d = (coef * W.sum(1)).astype(np.float32)
    G = (np.diag(d) - coef * W).astype(np.float32)  # feats_b = G @ x_b
    GT = np.ascontiguousarray(G.T).astype(ml_dtypes.bfloat16).reshape(KC, 128, T)

    perm = np.concatenate([  # torch gate order i,f,g,o -> [g,i,f,o]
        np.arange(2 * H, 3 * H), np.arange(0, H),
        np.arange(H, 2 * H), np.arange(3 * H, 4 * H),
    ])
    bf = ml_dtypes.bfloat16
    # tanh-as-sigmoid folding: g-gate rows x2; h~=h/2 so h-consumers x2.
    gsc = np.ones((G4, 1), np.float32)
    gsc[0:H] = 2.0

    A0 = np.zeros((G4, NP), np.float32)
    A0[:, :N] = np.asarray(inputs["Wih0"], np.float32)[perm, :N] * gsc
    A0T = np.ascontiguousarray(A0.T).astype(bf).reshape(NC2, 128, G4)
    b0 = ((np.asarray(inputs["bih0"], np.float32)
           + np.asarray(inputs["bhh0"], np.float32))[perm] * gsc[:, 0])
    b0_t = np.ascontiguousarray(b0.reshape(GC, 128).T)
    Whh0T = np.ascontiguousarray(
        (np.asarray(inputs["Whh0"], np.float32)[perm] * gsc * 2.0).T
    ).astype(bf).reshape(KC, 128, G4)

    A1T = np.ascontiguousarray(
        (np.asarray(inputs["Wih1"], np.float32)[perm] * gsc * 2.0).T
    ).astype(bf).reshape(KC, 128, G4)
    b1 = ((np.asarray(inputs["bih1"], np.float32)
           + np.asarray(inputs["bhh1"], np.float32))[perm] * gsc[:, 0])
    b1_t = np.ascontiguousarray(b1.reshape(GC, 128).T)
    Whh1T = np.ascontiguousarray(
        (np.asarray(inputs["Whh1"], np.float32)[perm] * gsc * 2.0).T
    ).astype(bf).reshape(KC, 128, G4)

    WoutT = np.ascontiguousarray(
        (np.asarray(inputs["Wout"], np.float32) * 2.0).T
    ).astype(bf).reshape(KC, 128, OUT)
    bout_r = np.broadcast_to(
        np.asarray(inputs["bout"], np.float32), (PB, OUT)
    ).copy()

    ident = np.eye(128, dtype=np.float32).astype(bf)

    xp = np.zeros((B, T, NP), ml_dtypes.bfloat16)
    xp[:, :, :N] = x.astype(ml_dtypes.bfloat16)

    shared = dict(
        gt=GT, a0t=A0T, b0=b0_t, whh0t=Whh0T, a1t=A1T, b1=b1_t,
        whh1t=Whh1T, woutt=WoutT, boutr=bout_r, ident=ident,
    )
    in_maps = []
    for c in range(NCORES):
        m = dict(shared)
        m["x"] = np.ascontiguousarray(xp[c * PB:(c + 1) * PB])
        in_maps.append(m)
    return in_maps


def kernel(**inputs):
    nc = _get_nc()
    in_maps = _prep_host(inputs)
    res = run_bass_kernel_spmd(nc, in_maps, core_ids=list(range(NCORES)))
    out = np.concatenate([r["out"] for r in res.results], axis=0)
    return out.astype(np.float32)
